# revision 3
# baseline (speedup 1.0000x reference)
"""DCT-based 1.25x upsample (2D DCT-II -> zero-pad spectrum -> 2D IDCT).

The whole reference computation is linear per (b, c) slice:
    out = M @ x @ M^T,   M = E960[:, :768] @ D768  (960x768, precomputed on host)
where D is the DCT-II matrix (norm=None) and E the IDCT matrix; zero-padding
the spectrum is folded into taking the first 768 columns of E.

On each NeuronCore (data-parallel over batch: 16 batches / 8 cores = 2 each,
x 3 channels = 6 slices per core) we run two chained matmuls per slice with
the tensor engine's `out = lhsT.T @ rhs` convention and the shared constant
Mt = M^T (768x960) as the moving operand:
    step 1:  W   = matmul(lhsT=x,  rhs=Mt) = x^T @ M^T         [768, 960]
    step 2:  out = matmul(lhsT=W,  rhs=Mt) = M @ x @ M^T       [960, 960]
W comes out of step 1 in PSUM with exactly the [K-partition, free] layout
step 2 needs for lhsT, so no transposes anywhere.

Matmuls run as float32r (fp32 bits, FP22 multiplies, fp32 accumulate):
1 PE cycle/row vs 4 for true fp32, end-to-end rel err ~1e-4.
"""

import numpy as np

import concourse.bass as bass  # noqa: F401  (engine types route via nc)
import concourse.mybir as mybir
import concourse.tile as tile
from concourse import bacc
from concourse.bass_utils import run_bass_kernel_spmd

# Problem shape (hardcoded per contract)
B, C, H = 16, 3, 768
OUT = 960  # H * 1.25
N_CORES = 8
SLICES = (B * C) // N_CORES  # 6 per core

P = 128
KT = H // P          # 6 contraction tiles
MT1 = H // P         # 6 output-row tiles for step 1 (x columns)
M2 = 120
MT2 = OUT // M2      # 8 output-row tiles for step 2
NT = 2
NW = OUT // NT       # 480-wide moving chunks (<= 512 fp32 PSUM bank)

MM_DT = mybir.dt.float32r  # set to mybir.dt.float32 for full-precision (4x slower)


def _build_mt() -> np.ndarray:
    """Mt = (E960[:, :768] @ D768)^T as float32, computed in float64."""
    n = np.arange(H, dtype=np.float64)
    k = np.arange(H, dtype=np.float64)[:, None]
    D = 2.0 * np.cos(np.pi * (2.0 * n[None, :] + 1.0) * k / (2.0 * H))

    n2 = np.arange(OUT, dtype=np.float64)[:, None]
    k2 = np.arange(OUT, dtype=np.float64)[None, :]
    E = np.cos(np.pi * (2.0 * n2 + 1.0) * k2 / (2.0 * OUT)) / OUT
    E[:, 0] = 1.0 / (2.0 * OUT)

    M = E[:, :H] @ D  # [960, 768]
    return np.ascontiguousarray(M.T).astype(np.float32)  # [768, 960]


def _build_program():
    nc = bacc.Bacc(None, target_bir_lowering=False, debug=False)

    x_ext = nc.dram_tensor("x", [SLICES, H, H], MM_DT, kind="ExternalInput")
    mt_ext = nc.dram_tensor("mt", [H, OUT], MM_DT, kind="ExternalInput")
    out_ext = nc.dram_tensor(
        "out", [SLICES, OUT, OUT], mybir.dt.float32, kind="ExternalOutput"
    )

    with tile.TileContext(nc) as tc:
        with (
            tc.tile_pool(name="const", bufs=1) as const_pool,
            tc.tile_pool(name="xp", bufs=2) as x_pool,
            tc.tile_pool(name="wp", bufs=2) as w_pool,
            tc.tile_pool(name="op", bufs=4) as o_pool,
            tc.tile_pool(name="ps", bufs=8, space="PSUM") as psum_pool,
        ):
            # Shared constant Mt, striped K-on-partitions: mt_sb[p, ko, n] = Mt[ko*P+p, n]
            mt_sb = const_pool.tile([P, KT, OUT], MM_DT)
            nc.sync.dma_start(mt_sb[:], mt_ext[:].rearrange("(ko p) n -> p ko n", p=P))
            mt_r = mt_sb[:]

            for s in range(SLICES):
                # x slice, rows striped onto partitions
                x_sb = x_pool.tile([P, KT, H], MM_DT)
                nc.sync.dma_start(
                    x_sb[:], x_ext[s].rearrange("(ko p) j -> p ko j", p=P)
                )
                x_r = x_sb[:]

                # Step 1: W = x^T @ Mt, K-striped for step 2:
                # w_sb[p, m, l] = W[m*P + p, l]
                w_sb = w_pool.tile([P, KT, OUT], MM_DT)
                for m in range(MT1):
                    psums = [
                        psum_pool.tile([P, NW], mybir.dt.float32, tag="ps", name=f"ps{n}")
                        for n in range(NT)
                    ]
                    for k in range(KT):
                        for n in range(NT):
                            nc.tensor.matmul(
                                psums[n][:],
                                x_r[:, k, m * P : (m + 1) * P],
                                mt_r[:, k, n * NW : (n + 1) * NW],
                                start=(k == 0),
                                stop=(k == KT - 1),
                            )
                    for n in range(NT):
                        nc.vector.tensor_copy(
                            w_sb[:, m, n * NW : (n + 1) * NW], psums[n][:]
                        )
                w_r = w_sb[:]

                # Step 2: out = W^T @ Mt
                for m in range(MT2):
                    psums = [
                        psum_pool.tile([P, NW], mybir.dt.float32, tag="ps", name=f"ps{n}")
                        for n in range(NT)
                    ]
                    o_sb = o_pool.tile([M2, OUT], mybir.dt.float32)
                    for k in range(KT):
                        for n in range(NT):
                            nc.tensor.matmul(
                                psums[n][:M2, :],
                                w_r[:, k, m * M2 : (m + 1) * M2],
                                mt_r[:, k, n * NW : (n + 1) * NW],
                                start=(k == 0),
                                stop=(k == KT - 1),
                            )
                    for n in range(NT):
                        nc.vector.tensor_copy(
                            o_sb[:, n * NW : (n + 1) * NW], psums[n][:M2, :]
                        )
                    nc.sync.dma_start(out_ext[s, m * M2 : (m + 1) * M2, :], o_sb[:])

    nc.compile()
    return nc


_CACHE: dict = {}


def _get_program():
    if "nc" not in _CACHE:
        _CACHE["nc"] = _build_program()
        _CACHE["mt"] = _build_mt()
    return _CACHE["nc"], _CACHE["mt"]


def kernel(x: np.ndarray, _trace: bool = False):
    assert x.shape == (B, C, H, H), x.shape
    nc, mt = _get_program()
    x = np.ascontiguousarray(x, dtype=np.float32)
    per_core = B // N_CORES
    in_maps = [
        {
            "x": x[i * per_core : (i + 1) * per_core].reshape(SLICES, H, H),
            "mt": mt,
        }
        for i in range(N_CORES)
    ]
    res = run_bass_kernel_spmd(nc, in_maps, list(range(N_CORES)), trace=_trace)
    out = np.empty((B, C, OUT, OUT), dtype=np.float32)
    for i in range(N_CORES):
        out[i * per_core : (i + 1) * per_core] = res.results[i]["out"].reshape(
            per_core, C, OUT, OUT
        )
    if _trace:
        return out, res
    return out


# revision 6
# speedup vs baseline: 1.0150x; 1.0150x over previous
"""DCT-based 1.25x upsample (2D DCT-II -> zero-pad spectrum -> 2D IDCT).

The whole reference computation is linear per (b, c) slice:
    out = M @ x @ M^T,   M = E960[:, :768] @ D768  (960x768, precomputed on host)
where D is the DCT-II matrix (norm=None) and E the IDCT matrix; zero-padding
the spectrum is folded into taking the first 768 columns of E.

On each NeuronCore (data-parallel over batch: 16 batches / 8 cores = 2 each,
x 3 channels = 6 slices per core) we run two chained matmuls per slice with
the tensor engine's `out = lhsT.T @ rhs` convention and the shared constant
Mt = M^T (768x960) as the moving operand:
    step 1:  W   = matmul(lhsT=x,  rhs=Mt) = x^T @ M^T         [768, 960]
    step 2:  out = matmul(lhsT=W,  rhs=Mt) = M @ x @ M^T       [960, 960]
W comes out of step 1 in PSUM with exactly the [K-partition, free] layout
step 2 needs for lhsT, so no transposes anywhere.

Matmuls run as float32r (fp32 bits, FP22 multiplies, fp32 accumulate):
1 PE cycle/row vs 4 for true fp32, end-to-end rel err ~1e-4.
"""

import numpy as np

import concourse.bass as bass  # noqa: F401  (engine types route via nc)
import concourse.mybir as mybir
import concourse.tile as tile
from concourse import bacc
from concourse.bass_utils import run_bass_kernel_spmd

# Problem shape (hardcoded per contract)
B, C, H = 16, 3, 768
OUT = 960  # H * 1.25
N_CORES = 8
SLICES = (B * C) // N_CORES  # 6 per core

P = 128
KT = H // P          # 6 contraction tiles
MT1 = H // P         # 6 output-row tiles for step 1 (x columns)
M2 = 120
MT2 = OUT // M2      # 8 output-row tiles for step 2
NT = 2
NW = OUT // NT       # 480-wide moving chunks (<= 512 fp32 PSUM bank)

MM_DT = mybir.dt.float32r  # set to mybir.dt.float32 for full-precision (4x slower)


def _build_mt() -> np.ndarray:
    """Mt = (E960[:, :768] @ D768)^T as float32, computed in float64."""
    n = np.arange(H, dtype=np.float64)
    k = np.arange(H, dtype=np.float64)[:, None]
    D = 2.0 * np.cos(np.pi * (2.0 * n[None, :] + 1.0) * k / (2.0 * H))

    n2 = np.arange(OUT, dtype=np.float64)[:, None]
    k2 = np.arange(OUT, dtype=np.float64)[None, :]
    E = np.cos(np.pi * (2.0 * n2 + 1.0) * k2 / (2.0 * OUT)) / OUT
    E[:, 0] = 1.0 / (2.0 * OUT)

    M = E[:, :H] @ D  # [960, 768]
    return np.ascontiguousarray(M.T).astype(np.float32)  # [768, 960]


def _build_program():
    nc = bacc.Bacc(None, target_bir_lowering=False, debug=False)

    x_ext = nc.dram_tensor("x", [SLICES, H, H], MM_DT, kind="ExternalInput")
    mt_ext = nc.dram_tensor("mt", [H, OUT], MM_DT, kind="ExternalInput")
    out_ext = nc.dram_tensor(
        "out", [SLICES, OUT, OUT], mybir.dt.float32, kind="ExternalOutput"
    )

    with tile.TileContext(nc) as tc:
        with (
            tc.tile_pool(name="const", bufs=1) as const_pool,
            tc.tile_pool(name="xp", bufs=2) as x_pool,
            tc.tile_pool(name="wp", bufs=2) as w_pool,
            tc.tile_pool(name="op", bufs=4) as o_pool,
            tc.tile_pool(name="ps", bufs=8, space="PSUM") as psum_pool,
        ):
            # Shared constant Mt, striped K-on-partitions: mt_sb[p, ko, n] = Mt[ko*P+p, n]
            # Loaded stripe-by-stripe so the first matmuls only wait on stripe 0.
            mt_sb = const_pool.tile([P, KT, OUT], MM_DT)
            mt_dram = mt_ext[:].rearrange("(ko p) n -> p ko n", p=P)
            for k in range(KT):
                nc.sync.dma_start(mt_sb[:, k, :], mt_dram[:, k, :])
            mt_r = mt_sb[:]

            for s in range(SLICES):
                # x slice, rows striped onto partitions
                x_sb = x_pool.tile([P, KT, H], MM_DT)
                x_dram = x_ext[s].rearrange("(ko p) j -> p ko j", p=P)
                for k in range(KT):
                    nc.sync.dma_start(x_sb[:, k, :], x_dram[:, k, :])
                x_r = x_sb[:]

                # Step 1: W = x^T @ Mt, K-striped for step 2:
                # w_sb[p, m, l] = W[m*P + p, l]
                # k-outer over halves of m so first-slice matmuls start as soon
                # as stripe k=0 lands (6 live PSUM banks per half).
                w_sb = w_pool.tile([P, KT, OUT], MM_DT)
                MH = MT1 // 2
                for half in range(2):
                    psums = [
                        [
                            psum_pool.tile(
                                [P, NW], mybir.dt.float32, tag="ps", name=f"ps{ml}_{n}"
                            )
                            for n in range(NT)
                        ]
                        for ml in range(MH)
                    ]
                    for k in range(KT):
                        for ml in range(MH):
                            m = half * MH + ml
                            for n in range(NT):
                                nc.tensor.matmul(
                                    psums[ml][n][:],
                                    x_r[:, k, m * P : (m + 1) * P],
                                    mt_r[:, k, n * NW : (n + 1) * NW],
                                    start=(k == 0),
                                    stop=(k == KT - 1),
                                )
                    for ml in range(MH):
                        m = half * MH + ml
                        for n in range(NT):
                            nc.vector.tensor_copy(
                                w_sb[:, m, n * NW : (n + 1) * NW], psums[ml][n][:]
                            )
                w_r = w_sb[:]

                # Step 2: out = W^T @ Mt
                for m in range(MT2):
                    psums = [
                        psum_pool.tile([P, NW], mybir.dt.float32, tag="ps", name=f"ps{n}")
                        for n in range(NT)
                    ]
                    o_sb = o_pool.tile([M2, OUT], mybir.dt.float32)
                    for k in range(KT):
                        for n in range(NT):
                            nc.tensor.matmul(
                                psums[n][:M2, :],
                                w_r[:, k, m * M2 : (m + 1) * M2],
                                mt_r[:, k, n * NW : (n + 1) * NW],
                                start=(k == 0),
                                stop=(k == KT - 1),
                            )
                    for n in range(NT):
                        nc.vector.tensor_copy(
                            o_sb[:, n * NW : (n + 1) * NW], psums[n][:M2, :]
                        )
                    nc.sync.dma_start(out_ext[s, m * M2 : (m + 1) * M2, :], o_sb[:])

    nc.compile()
    return nc


_CACHE: dict = {}


def _get_program():
    if "nc" not in _CACHE:
        _CACHE["nc"] = _build_program()
        _CACHE["mt"] = _build_mt()
    return _CACHE["nc"], _CACHE["mt"]


def kernel(x: np.ndarray, _trace: bool = False):
    assert x.shape == (B, C, H, H), x.shape
    nc, mt = _get_program()
    x = np.ascontiguousarray(x, dtype=np.float32)
    per_core = B // N_CORES
    in_maps = [
        {
            "x": x[i * per_core : (i + 1) * per_core].reshape(SLICES, H, H),
            "mt": mt,
        }
        for i in range(N_CORES)
    ]
    res = run_bass_kernel_spmd(nc, in_maps, list(range(N_CORES)), trace=_trace)
    out = np.empty((B, C, OUT, OUT), dtype=np.float32)
    for i in range(N_CORES):
        out[i * per_core : (i + 1) * per_core] = res.results[i]["out"].reshape(
            per_core, C, OUT, OUT
        )
    if _trace:
        return out, res
    return out
